# revision 39
# baseline (speedup 1.0000x reference)
"""Trainium2 Bass kernel for the ButterflyModule problem.

Semantics (N=4096 rows, B=8192 cols):
  x = data[indices_in]
  4 Givens-rotation butterfly layers (strides 1,2,4,8 within 16-row blocks)
  bias + smooth-ReLU on rows with (row%16)<8
  4 more butterfly layers (strides 1,2,4,8)
  out = data with rows idx_out replaced by the result

Device strategy: the 4 input layers compose into a dense 16x16 matrix per
16-row block (256 blocks), same for the 4 output layers.  Each 128-row group
is then one block-diagonal 128x128 matmul on the TensorEngine.  The
activation folds into per-partition scalars:

  y' = D.Min @ x + D.b          (D = diag(0.5 on act rows, 1 elsewhere))
  u  = m * y'                   (m = 1 on act rows, 0 elsewhere; ACT Square scale)
  s  = sqrt(u^2 + (0.05)^2 * m) (ACT Sqrt with per-partition bias)
  z  = y' + s                   (act rows: 0.5*(xa+sqrt(xa^2+0.01)); else y)
  out = Mout @ z

Rows are sharded across the 8 cores (512 rows each); rotations never cross
16-row block boundaries so there is no cross-core communication.
"""

import sys

if "/opt/trn_rl_repo" not in sys.path:
    sys.path.insert(0, "/opt/trn_rl_repo")

import numpy as np

N_ROWS = 4096
N_COLS = 8192
COL_BLOCK = 16
NUM_ACT = 8
CURVATURE = 0.1
N_CORES = 8
ROWS_PER_CORE = N_ROWS // N_CORES          # 512
GROUPS_PER_CORE = ROWS_PER_CORE // 128     # 4
FREE = 512                                 # matmul moving-dim tile (fp32 max)
N_FTILES = N_COLS // FREE                  # 16

_PROGRAM_CACHE = {}


def _butterfly_mats(angles64):
    """Compose butterfly layers into per-block 16x16 matrices.

    angles64: [8, 2048] float64.  Returns (Min, Mout) each [256, 16, 16],
    where layer l uses stride 1<<(l%4) and block b uses angles[l, 8b:8b+8]
    ordered by the low row index within the block.
    """
    nb = N_ROWS // COL_BLOCK

    def accum(l0, l1):
        G = np.broadcast_to(np.eye(COL_BLOCK), (nb, COL_BLOCK, COL_BLOCK)).copy()
        for l in range(l0, l1):
            stride = 1 << (l % 4)
            offs = [o for o in range(COL_BLOCK) if (o & stride) == 0]
            a = angles64[l].reshape(nb, NUM_ACT)
            c = np.cos(a)
            s = np.sin(a)
            for k, o in enumerate(offs):
                gl = G[:, o, :].copy()
                gh = G[:, o + stride, :].copy()
                G[:, o, :] = c[:, k, None] * gl + s[:, k, None] * gh
                G[:, o + stride, :] = -s[:, k, None] * gl + c[:, k, None] * gh
        return G

    return accum(0, 4), accum(4, 8)


def _host_weights(angles, biases):
    """Build per-core weight tensors for the device kernel."""
    ang64 = np.asarray(angles, np.float64)
    b64 = np.asarray(biases, np.float64)
    Min, Mout = _butterfly_mats(ang64)

    off = np.arange(COL_BLOCK)
    d16 = np.where(off < NUM_ACT, 0.5, 1.0)          # post-layer-4 scale
    Minp = Min * d16[None, :, None]                  # diag(d) @ Min (row scale)

    MinT = Minp.transpose(0, 2, 1)                   # per-block lhsT
    MoutT = Mout.transpose(0, 2, 1)

    def block_diag8(blocksT):
        out = np.zeros((128, 128))
        for i in range(8):
            out[i * 16:(i + 1) * 16, i * 16:(i + 1) * 16] = blocksT[i]
        return out

    # bias' = d * bias on act rows, 0 elsewhere, laid out over all 4096 rows
    bf = np.zeros(N_ROWS)
    blk = np.arange(N_ROWS // COL_BLOCK)
    for o in range(NUM_ACT):
        bf[blk * COL_BLOCK + o] = 0.5 * b64[blk * NUM_ACT + o]

    p = np.arange(128)
    act = (p % COL_BLOCK) < NUM_ACT
    m_vec = np.where(act, 1.0, 0.0).astype(np.float32).reshape(128, 1)
    sqb_vec = np.where(act, (0.5 * CURVATURE) ** 2, 0.0).astype(np.float32)
    sqb_vec = sqb_vec.reshape(128, 1)

    per_core = []
    for c in range(N_CORES):
        win = np.zeros((GROUPS_PER_CORE, 128, 128))
        wout = np.zeros((GROUPS_PER_CORE, 128, 128))
        for g in range(GROUPS_PER_CORE):
            g_glob = c * GROUPS_PER_CORE + g
            win[g] = block_diag8(MinT[g_glob * 8:(g_glob + 1) * 8])
            wout[g] = block_diag8(MoutT[g_glob * 8:(g_glob + 1) * 8])
        win_dram = win.transpose(1, 0, 2).reshape(128, GROUPS_PER_CORE * 128)
        wout_dram = wout.transpose(1, 0, 2).reshape(128, GROUPS_PER_CORE * 128)
        bias_dram = (
            bf[c * ROWS_PER_CORE:(c + 1) * ROWS_PER_CORE]
            .reshape(GROUPS_PER_CORE, 128)
            .T
        )
        per_core.append(
            {
                "win": np.ascontiguousarray(win_dram, dtype=np.float32),
                "wout": np.ascontiguousarray(wout_dram, dtype=np.float32),
                "biasv": np.ascontiguousarray(bias_dram, dtype=np.float32),
                "mvec": m_vec,
                "sqbv": sqb_vec,
            }
        )
    return per_core


def _build_program(reps=None, use_f32r=None, mode=None, xbufs=None,
                   wbufs=None, pipelined=None):
    import os

    import concourse.bacc as bacc
    import concourse.mybir as mybir
    from concourse.tile import TileContext

    f32 = mybir.dt.float32
    f32r = mybir.dt.float32r
    AFT = mybir.ActivationFunctionType
    Alu = mybir.AluOpType
    if use_f32r is None:
        use_f32r = os.environ.get("BUTTERFLY_FP32R", "0") == "1"
    if reps is None:
        reps = int(os.environ.get("BUTTERFLY_REPS", "1"))
    if mode is None:
        mode = os.environ.get("BUTTERFLY_MODE", "full")  # full|dma|compute
    if xbufs is None:
        xbufs = int(os.environ.get("BUTTERFLY_XBUFS", "3"))
    if wbufs is None:
        wbufs = int(os.environ.get("BUTTERFLY_WBUFS", "4"))
    if pipelined is None:
        pipelined = os.environ.get("BUTTERFLY_PIPE", "1") == "1"
    pybufs = int(os.environ.get("BUTTERFLY_PYBUFS", "2"))
    pobufs = int(os.environ.get("BUTTERFLY_POBUFS", "2"))
    odma = os.environ.get("BUTTERFLY_ODMA", "sp")  # sp | act | pool
    wtile = int(os.environ.get("BUTTERFLY_W", "1024"))

    W = 1024                    # megatile width (2 PSUM banks)
    n_wtiles = N_COLS // W      # 8 per row-group

    fmm = f32r if use_f32r else f32

    def mm_cast(ap):
        return ap

    nc = bacc.Bacc("TRN2", target_bir_lowering=False)
    x = nc.dram_tensor("x", [ROWS_PER_CORE, N_COLS], fmm, kind="ExternalInput")
    win = nc.dram_tensor("win", [128, GROUPS_PER_CORE * 128], fmm,
                         kind="ExternalInput")
    wout = nc.dram_tensor("wout", [128, GROUPS_PER_CORE * 128], fmm,
                          kind="ExternalInput")
    biasv = nc.dram_tensor("biasv", [128, GROUPS_PER_CORE], f32,
                           kind="ExternalInput")
    mvec = nc.dram_tensor("mvec", [128, 1], f32, kind="ExternalInput")
    sqbv = nc.dram_tensor("sqbv", [128, 1], f32, kind="ExternalInput")
    yout = nc.dram_tensor("yout", [ROWS_PER_CORE, N_COLS], f32,
                          kind="ExternalOutput")

    with TileContext(nc) as tc:
        with (
            tc.tile_pool(name="consts", bufs=1) as cpool,
            tc.tile_pool(name="xin", bufs=xbufs) as xpool,
            tc.tile_pool(name="work", bufs=wbufs) as wpool,
            tc.tile_pool(name="psum_y", bufs=pybufs, space="PSUM") as pypool,
            tc.tile_pool(name="psum_o", bufs=pobufs, space="PSUM") as popool,
        ):
            win_sb = cpool.tile([128, GROUPS_PER_CORE * 128], fmm)
            wout_sb = cpool.tile([128, GROUPS_PER_CORE * 128], fmm)
            bias_sb = cpool.tile([128, GROUPS_PER_CORE], f32)
            m_sb = cpool.tile([128, 1], f32)
            sqb_sb = cpool.tile([128, 1], f32)
            # group-0 weights first so the first matmul can start early;
            # remaining groups stream in behind the first x tiles.
            g0 = slice(0, 128)
            nc.sync.dma_start(win_sb[:, g0], win[:, g0])
            nc.sync.dma_start(wout_sb[:, g0], wout[:, g0])
            nc.sync.dma_start(bias_sb[:], biasv[:])
            nc.sync.dma_start(m_sb[:], mvec[:])
            nc.sync.dma_start(sqb_sb[:], sqbv[:])
            grest = slice(128, GROUPS_PER_CORE * 128)
            nc.scalar.dma_start(win_sb[:, grest], win[:, grest])
            nc.scalar.dma_start(wout_sb[:, grest], wout[:, grest])

            import contextlib

            stag = os.environ.get("BUTTERFLY_STAG", "0") == "1"
            loop_cm = (tc.For_i(0, reps, 1, staggered_reset=stag)
                       if reps > 1 else contextlib.nullcontext())
            with loop_cm:
                if mode == "full":
                    _emit_body(nc, tc, mybir, x, yout, win_sb, wout_sb,
                               bias_sb, m_sb, sqb_sb, xpool, wpool, pypool,
                               popool, fmm, pipelined=pipelined, odma=odma,
                               W=wtile)
                elif mode == "tiny":
                    xt = xpool.tile([128, 1024], fmm, name="xt")
                    nc.sync.dma_start(xt[:], x[0:128, 0:1024])
                    nc.sync.dma_start(yout[0:128, 0:1024], xt[:])
                elif mode == "dma":
                    W = 1024
                    for g in range(GROUPS_PER_CORE):
                        rows = slice(g * 128, (g + 1) * 128)
                        for j in range(N_COLS // W):
                            cols = slice(j * W, (j + 1) * W)
                            xt = xpool.tile([128, W], fmm, name="xt")
                            nc.sync.dma_start(xt[:], x[rows, cols])
                            nc.sync.dma_start(yout[rows, cols], xt[:])
                elif mode == "dmaflat":
                    xf = x[:].flatten().rearrange(
                        "(n p c) -> n p c", p=128, c=1024)
                    yf = yout[:].flatten().rearrange(
                        "(n p c) -> n p c", p=128, c=1024)
                    for i in range(xf.shape[0]):
                        xt = xpool.tile([128, 1024], fmm, name="xt")
                        nc.sync.dma_start(xt[:], xf[i])
                        nc.sync.dma_start(yf[i], xt[:])
                elif mode == "dmabig":
                    for g in range(GROUPS_PER_CORE):
                        rows = slice(g * 128, (g + 1) * 128)
                        xb = xpool.tile([128, N_COLS], fmm, name="xb",
                                        bufs=2)
                        nc.sync.dma_start(xb[:], x[rows, :])
                        nc.sync.dma_start(yout[rows, :], xb[:])
                elif mode == "compute":
                    xc = cpool.tile([128, 1024], fmm, name="xc")
                    nc.vector.memset(xc[:], 1.0)
                    _emit_body(nc, tc, mybir, None, None, win_sb, wout_sb,
                               bias_sb, m_sb, sqb_sb, None, wpool, pypool,
                               popool, fmm, xc=xc)
                elif mode == "inpe":
                    # in-DMA + stage-1 MMs + PSUM evac only
                    for g in range(GROUPS_PER_CORE):
                        lhs1 = win_sb[:, g * 128:(g + 1) * 128]
                        for j in range(N_COLS // 1024):
                            xt = xpool.tile([128, 1024], fmm, name="xt")
                            nc.sync.dma_start(
                                xt[:], x[g * 128:(g + 1) * 128,
                                         j * 1024:(j + 1) * 1024])
                            py = pypool.tile([128, 1024], f32, name="py")
                            for h in range(2):
                                cs = slice(h * 512, (h + 1) * 512)
                                nc.tensor.matmul(py[:, cs], lhs1, xt[:, cs],
                                                 start=True, stop=True)
                            yt = wpool.tile([128, 1024], f32, name="yt")
                            nc.scalar.activation(
                                yt[:], py[:],
                                mybir.ActivationFunctionType.Identity,
                                bias=bias_sb[:, g:g + 1], scale=1.0)
                elif mode == "indep":
                    # compute from a memset tile + unconsumed in-DMAs:
                    # isolates DMA-write/engine contention from deps
                    xc = cpool.tile([128, 1024], fmm, name="xc")
                    nc.vector.memset(xc[:], 1.0)
                    for g in range(GROUPS_PER_CORE):
                        for j in range(N_COLS // 1024):
                            dummy = xpool.tile([128, 1024], fmm,
                                               name="dummy")
                            nc.sync.dma_start(
                                dummy[:],
                                x[g * 128:(g + 1) * 128,
                                  j * 1024:(j + 1) * 1024])
                    _emit_body(nc, tc, mybir, None, None, win_sb, wout_sb,
                               bias_sb, m_sb, sqb_sb, None, wpool, pypool,
                               popool, fmm, xc=xc, pipelined=pipelined,
                               odma=odma)
                elif mode == "noout":
                    _emit_body(nc, tc, mybir, x, None, win_sb, wout_sb,
                               bias_sb, m_sb, sqb_sb, xpool, wpool, pypool,
                               popool, fmm, pipelined=pipelined, odma=odma)
                elif mode == "noin":
                    xc = cpool.tile([128, 1024], fmm, name="xc")
                    nc.vector.memset(xc[:], 1.0)
                    _emit_body(nc, tc, mybir, None, yout, win_sb, wout_sb,
                               bias_sb, m_sb, sqb_sb, None, wpool, pypool,
                               popool, fmm, xc=xc, pipelined=pipelined,
                               odma=odma)

    nc.compile()
    return nc


def _emit_body(nc, tc, mybir, x, yout, win_sb, wout_sb, bias_sb, m_sb, sqb_sb,
               xpool, wpool, pypool, popool, fmm, xc=None, pipelined=True,
               odma="sp", W=1024):
    f32 = mybir.dt.float32
    AFT = mybir.ActivationFunctionType
    Alu = mybir.AluOpType
    n_wtiles = N_COLS // W

    # Software-pipelined: stage 2 of megatile k-1 is emitted after the
    # elementwise chain of megatile k, so PE never waits on z.
    tiles = [(g, j) for g in range(GROUPS_PER_CORE) for j in range(n_wtiles)]
    pending = None  # (g, j, zt)
    it = 0
    out_eng = {"sp": nc.sync, "act": nc.scalar, "pool": nc.gpsimd}[odma]

    def stage2(g, j, zt, it):
        lhs2 = wout_sb[:, g * 128:(g + 1) * 128]
        po = popool.tile([128, W], f32, name="po")
        for h in range(W // FREE):
            cs = slice(h * FREE, (h + 1) * FREE)
            nc.tensor.matmul(po[:, cs], lhs2, zt[:, cs],
                             start=True, stop=True)
        ot = wpool.tile([128, W], f32, name="ot")
        if it % 2 == 0:
            nc.vector.tensor_copy(ot[:], po[:])
        else:
            nc.scalar.copy(ot[:], po[:])
        if yout is not None:
            out_eng.dma_start(
                yout[g * 128:(g + 1) * 128, j * W:(j + 1) * W], ot[:])

    for (g, j) in tiles:
        rows = slice(g * 128, (g + 1) * 128)
        cols = slice(j * W, (j + 1) * W)
        lhs1 = win_sb[:, g * 128:(g + 1) * 128]
        bias_g = bias_sb[:, g:g + 1]
        if xc is not None:
            xt = xc
        else:
            xt = xpool.tile([128, W], fmm, name="xt")
            nc.sync.dma_start(xt[:], x[rows, cols])

        # stage 1: y' = diag(d).Min @ x + d*b   (one MM per PSUM bank)
        py = pypool.tile([128, W], f32, name="py")
        for h in range(W // FREE):
            cs = slice(h * FREE, (h + 1) * FREE)
            nc.tensor.matmul(py[:, cs], lhs1, xt[:, cs],
                             start=True, stop=True)
        yt = wpool.tile([128, W], f32, name="yt")
        nc.scalar.activation(yt[:], py[:], AFT.Identity,
                             bias=bias_g, scale=1.0)

        # t2 = y'^2 ; s = sqrt(m*t2 + (m*0.05)^2) ; z = y' + s
        tt = wpool.tile([128, W], f32, name="tt")
        nc.vector.tensor_tensor(tt[:], yt[:], yt[:], Alu.mult)
        st = wpool.tile([128, W], f32, name="st")
        nc.scalar.activation(st[:], tt[:], AFT.Sqrt,
                             bias=sqb_sb[:, 0:1],
                             scale=m_sb[:, 0:1])
        zt = wpool.tile([128, W], fmm, name="zt")
        nc.vector.tensor_tensor(zt[:], yt[:], st[:], Alu.add)

        if not pipelined:
            stage2(g, j, zt, it)
            it += 1
        else:
            if pending is not None:
                stage2(*pending, it)
                it += 1
            pending = (g, j, zt)

    if pending is not None:
        stage2(*pending, it)


def _get_program():
    if "nc" not in _PROGRAM_CACHE:
        _PROGRAM_CACHE["nc"] = _build_program()
    return _PROGRAM_CACHE["nc"]


def kernel(data, angles, biases, indices_in, idx_out, _return_results=False):
    from concourse import bass_utils

    data = np.asarray(data)
    x_full = np.ascontiguousarray(
        np.asarray(data, np.float32)[np.asarray(indices_in)]
    )
    weights = _host_weights(angles, biases)
    in_maps = []
    for c in range(N_CORES):
        im = dict(weights[c])
        im["x"] = np.ascontiguousarray(
            x_full[c * ROWS_PER_CORE:(c + 1) * ROWS_PER_CORE]
        )
        in_maps.append(im)

    nc = _get_program()
    res = bass_utils.run_bass_kernel_spmd(nc, in_maps,
                                          core_ids=list(range(N_CORES)))
    y = np.concatenate([res.results[c]["yout"] for c in range(N_CORES)], axis=0)
    out = np.array(data, copy=True)
    out[np.asarray(idx_out)] = y
    if _return_results:
        return out, res
    return out
